# revision 15
# baseline (speedup 1.0000x reference)
"""Trainium2 kernel for nn_ClauseFunction (segment_reduce):
C[b,g] = softor_s(softand_l(x[b, I_i[g,s,l]])), gamma=1e-3.

Since gamma is tiny, softand/softor are within gamma*ln(L|S) <= 0.0035 of hard
min/max, so compute min_l then max_s on uint8-quantized values (round(x*255),
quantization error 1/510; measured rel err 3.3e-3 vs the 2e-2 gate).

Strategy: shard over G (each of 8 cores handles 256 g-columns). Per core the
x table is packed [128 part, 2048 col, 4 batch] uint8 -- partition p holds
batches 4*(p%16)..+3, so each 16-partition group carries a full copy of x.
gpsimd ap_gather (SBUF->SBUF, per-group index lists) gathers 8192 indices per
group (one 32-column slice of g per group) in tapered chunks. DVE does
pairwise-min over l (3 passes) and pairwise-max over s (4+1 passes) per chunk (chunks are g-aligned so the s-max folds into each chunk); Act
converts finished g-columns to f32. The ucode library is loaded explicitly up
front and the table broadcast is split across DMA queues so the first gather
starts early.
"""

import numpy as np

import concourse.bacc as bacc
import concourse.bass as bass
import concourse.tile as tile
from concourse import library_config, mybir
from concourse.bass_utils import run_bass_kernel_spmd

B, G, S, L = 64, 2048, 32, 8
NCORES = 8
GSH = G // NCORES  # 256 g per core
NGRP = 8  # gpsimd cores / 16-partition groups
GPG = GSH // NGRP  # 32 g-columns per group
IDX_PER_GRP = GPG * S * L  # 8192
# tapered chunk sizes (indices per group per call); each is a multiple of 256
# (one g-column) so the s-max completes within the chunk
CHUNKS = [2048, 2048, 2048, 1536, 512]
assert sum(CHUNKS) == IDX_PER_GRP

_nc_cache = None
last_result = None


def _v(t, dims, off=0):
    """View of tile t with explicit free-dim [stride, count] pairs (elements)."""
    return bass.AP(tensor=t.tensor, offset=t.offset + off, ap=[list(t.ap[0])] + dims)


def _build_nc():
    u8 = mybir.dt.uint8
    f32 = mybir.dt.float32
    nc = bacc.Bacc("TRN2", target_bir_lowering=False)
    # xtab[q, j*4+d] = x[4q+d, j] (uint8 batch-packed table, one copy)
    xtab_in = nc.dram_tensor("xtab", [16, G * 4], u8, kind="ExternalInput")
    # idx[16c+q, t] = list position t*16+q of group c (int16 col ids)
    idx_in = nc.dram_tensor(
        "idx", [128, IDX_PER_GRP // 16], mybir.dt.int16, kind="ExternalInput"
    )
    # c[16c+q, g''*4+d] = C[4q+d, 256*core + 32c + g'']
    c_out = nc.dram_tensor("c", [128, 128], f32, kind="ExternalOutput")

    with tile.TileContext(nc) as tc:
        with (
            tc.tile_pool(name="singles", bufs=1) as singles,
            tc.tile_pool(name="gath", bufs=2) as gath,
            tc.tile_pool(name="work", bufs=2) as work,
            tc.tile_pool(name="small", bufs=2) as small,
        ):
            nc.gpsimd.load_library(library_config.ap_gather)
            idxs = singles.tile([128, IDX_PER_GRP // 16], mybir.dt.int16)
            nc.scalar.dma_start(out=idxs, in_=idx_in[:, :])
            # broadcast the 16-partition table to all 8 groups; spread the
            # eighths across the three DMA-capable engines' queues so the
            # transfers run in parallel
            xt = singles.tile([128, G, 4], u8)
            engs = [nc.sync] * 3 + [nc.scalar] * 2 + [nc.gpsimd] * 3
            for i, eng in enumerate(engs):
                src = bass.AP(
                    tensor=xtab_in,
                    offset=i * 1024,
                    ap=[[0, NGRP], [G * 4, 16], [1, 1024]],
                )
                eng.dma_start(out=xt[:, i * 256 : (i + 1) * 256, :], in_=src)

            cf = singles.tile([128, 128], f32)
            off_c = 0  # cumulative index offset per group
            for ck, K in enumerate(CHUNKS):
                glc = K // 256  # g-columns per group this chunk
                goff = off_c // 256  # cumulative g-column offset
                gt = gath.tile([128, 2048, 4], u8, tag="gt")
                nc.gpsimd.ap_gather(
                    gt[:, :K, :],
                    xt[:, :, :],
                    idxs[:, off_c // 16 : (off_c + K) // 16],
                    channels=128,
                    num_elems=G,
                    d=4,
                    num_idxs=K,
                )
                # gt free offset = gl*1024 + s*32 + l*4 + d ; (gl,s) fuse: K/8
                m1 = work.tile([128, 256, 4, 4], u8, tag="m1")  # [gs, l(4), d]
                nc.vector.tensor_tensor(
                    out=_v(m1, [[16, K // 8], [4, 4], [1, 4]]),
                    in0=_v(gt, [[32, K // 8], [4, 4], [1, 4]]),
                    in1=_v(gt, [[32, K // 8], [4, 4], [1, 4]], 16),
                    op=mybir.AluOpType.min,
                )
                m2 = work.tile([128, 256, 2, 4], u8, tag="m2")  # [gs, l(2), d]
                nc.vector.tensor_tensor(
                    out=_v(m2, [[8, K // 8], [4, 2], [1, 4]]),
                    in0=_v(m1, [[16, K // 8], [4, 2], [1, 4]]),
                    in1=_v(m1, [[16, K // 8], [4, 2], [1, 4]], 8),
                    op=mybir.AluOpType.min,
                )
                vv = work.tile([128, 256, 4], u8, tag="vv")  # [gs, d] min over l
                nc.vector.tensor_tensor(
                    out=_v(vv, [[4, K // 8], [1, 4]]),
                    in0=_v(m2, [[8, K // 8], [1, 4]]),
                    in1=_v(m2, [[8, K // 8], [1, 4]], 4),
                    op=mybir.AluOpType.min,
                )
                # max over s within this chunk: vv offset = gl*128 + s*4 + d
                y1 = small.tile([128, 8, 16, 4], u8, tag="y1")
                nc.vector.tensor_tensor(
                    out=_v(y1, [[64, glc], [4, 16], [1, 4]]),
                    in0=_v(vv, [[128, glc], [4, 16], [1, 4]]),
                    in1=_v(vv, [[128, glc], [4, 16], [1, 4]], 64),
                    op=mybir.AluOpType.max,
                )
                y2 = small.tile([128, 8, 8, 4], u8, tag="y2")
                nc.vector.tensor_tensor(
                    out=_v(y2, [[32, glc], [4, 8], [1, 4]]),
                    in0=_v(y1, [[64, glc], [4, 8], [1, 4]]),
                    in1=_v(y1, [[64, glc], [4, 8], [1, 4]], 32),
                    op=mybir.AluOpType.max,
                )
                y3 = small.tile([128, 8, 4, 4], u8, tag="y3")
                nc.vector.tensor_tensor(
                    out=_v(y3, [[16, glc], [4, 4], [1, 4]]),
                    in0=_v(y2, [[32, glc], [4, 4], [1, 4]]),
                    in1=_v(y2, [[32, glc], [4, 4], [1, 4]], 16),
                    op=mybir.AluOpType.max,
                )
                y4 = small.tile([128, 8, 2, 4], u8, tag="y4")
                nc.vector.tensor_tensor(
                    out=_v(y4, [[8, glc], [4, 2], [1, 4]]),
                    in0=_v(y3, [[16, glc], [4, 2], [1, 4]]),
                    in1=_v(y3, [[16, glc], [4, 2], [1, 4]], 8),
                    op=mybir.AluOpType.max,
                )
                cg = small.tile([128, 8, 4], u8, tag="cg")
                nc.vector.tensor_tensor(
                    out=_v(cg, [[4, glc], [1, 4]]),
                    in0=_v(y4, [[8, glc], [1, 4]]),
                    in1=_v(y4, [[8, glc], [1, 4]], 4),
                    op=mybir.AluOpType.max,
                )
                nc.scalar.activation(
                    out=cf[:, goff * 4 : (goff + glc) * 4],
                    in_=cg[:, :glc, :],
                    func=mybir.ActivationFunctionType.Copy,
                    scale=1.0 / 255.0,
                )
                off_c += K
            nc.sync.dma_start(out=c_out[:, :], in_=cf)
    nc.finalize()
    return nc


def _prep_inputs(x: np.ndarray, I_i: np.ndarray):
    """Host-side layout: uint8 batch-packed table; per-core wrapped idx lists."""
    xs = np.clip(np.rint(x * 255.0), 0, 255).astype(np.uint8)  # [64, 2048]
    # xtab[q, j, d] = round(255*x[4q+d, j])
    xtab = np.ascontiguousarray(
        xs.reshape(16, 4, G).transpose(0, 2, 1)
    ).reshape(16, G * 4)
    I = np.asarray(I_i).astype(np.int16)  # [2048, 32, 8], values in [0, G)
    idx_maps = []
    for k in range(NCORES):
        Ik = I[k * GSH : (k + 1) * GSH]  # [256, 32, 8]
        flat = Ik.reshape(NGRP, IDX_PER_GRP)  # [c, j] with j=(g''*256+s*8+l)
        w = flat.reshape(NGRP, IDX_PER_GRP // 16, 16)  # [c, t, q]
        w = np.transpose(w, (0, 2, 1))  # [c, q, t]
        idx_maps.append(np.ascontiguousarray(w.reshape(128, IDX_PER_GRP // 16)))
    return xtab, idx_maps


def _unshard(results) -> np.ndarray:
    C = np.empty((B, G), dtype=np.float32)
    for k in range(NCORES):
        o = results[k]["c"].reshape(NGRP, 16, GPG, 4)  # [c, q, g'', d]
        C[:, k * GSH : (k + 1) * GSH] = o.transpose(1, 3, 0, 2).reshape(B, GSH)
    return C


def kernel(x: np.ndarray, I_i: np.ndarray) -> np.ndarray:
    global _nc_cache, last_result
    if _nc_cache is None:
        _nc_cache = _build_nc()
    nc = _nc_cache
    xtab, idx_maps = _prep_inputs(x, I_i)
    in_maps = [{"xtab": xtab, "idx": idx_maps[k]} for k in range(NCORES)]
    res = run_bass_kernel_spmd(nc, in_maps, core_ids=list(range(NCORES)))
    last_result = res
    return _unshard(res.results)
